# revision 13
# baseline (speedup 1.0000x reference)
"""Local windowed attention (window=128, look_backward=1, RoPE) on 8 TRN2 cores.

Sharding: data-parallel over batch (128 -> 16 per core).

Host prep (layout/dtype/embedding-preprocessing only -- all of the attention
itself, i.e. logits, causal-masked softmax and the weighted sum over values,
runs on device):
  * RoPE rotation applied to q,k on the host using global-position angles
    (rotation-invariance of dot products makes this exactly equivalent to the
    reference's window-relative angles); q,k uploaded pre-transposed d-major
    [64, N] bf16 with two batches stacked on the 128 SBUF partitions.
  * v blocked per window with a ones column appended (the softmax denominator
    then falls out of MM2's PSUM accumulation).

Device (the scalar engine's exp is the bottleneck: ACTIVATE costs
(cols+352)/1.2 ns, so the design minimizes scalar-queue columns+startups):
  * MM1 (logits^T, j-major): stationary k^T_w [64,128], moving [q^T_w|q^T_{w+1}]
    -> strip [cur_w | back_{w+1}]; batch halves on partition halves 0:63/64:127
    via tile_position (0,0)/(64,0) run concurrently on disjoint PE row-groups.
  * Chunks of 3 windows (1536 cols: [h0: 3 strips | h1: 3 strips]); 11 chunks
    per pair (last chunk holds windows 30,31 + junk).
  * STAGED chunks (first NSTG per pair): DVE evacuates S from PSUM to SBUF
    fp16 with the causal mask FUSED as an additive -30000 broadcast constant;
    one giant SBUF-sourced exp per pair then covers all staged columns with a
    single 352-cycle startup.  That exp is DEFERRED into the next pair's
    instruction stream so the scalar queue never waits on the evacuation.
  * DIRECT chunks: per-chunk exp straight from PSUM (scale=1/sqrt(D) folded
    into the activation); the 0/1 causal mask is applied post-exp on the
    otherwise-idle GPSIMD engine (SBUF-only: it cannot touch PSUM).
  * MM2 accumulates [back|cur] x v into PSUM groups (pop bufs=1: 2 banks,
    freeing 6 banks for double-buffered 3-bank S tiles); normalization =
    fast approximate reciprocal (~51 ULP) + one broadcast multiply per group
    on DVE.
  * Output written blocked [pos-in-window, (window, d)] bf16; host
    inverse-permutes and upcasts.
"""

import sys

sys.path.insert(0, "/opt/trn_rl_repo")

import numpy as np
import ml_dtypes

import concourse.bass as bass
import concourse.bacc as bacc
import concourse.mybir as mybir
import concourse.tile as tile
from concourse.bass_utils import run_bass_kernel_spmd

B, N, D, W = 128, 4096, 64, 128
NCORES = 8
BL = B // NCORES          # 16 batches per core
NP = BL // 2              # 8 batch-pairs per core
NW = N // W               # 32 windows
THETA = 10000.0
# chunking: 10 chunks of 3 windows + 1 chunk of 2 windows (+junk), all
# [128, 1536] PSUM tiles (3 banks; bufs=2 -> 6 banks, O pool takes 2)
NCH = 11
CW = 1536                 # chunk width in S / eh columns
NSTG = 2                  # staged chunks per pair (chunks 0..NSTG-1)
# MM2 window groups (start, len): 7-window groups fill a PSUM bank; the final
# windows are split into 2-window groups so little MM2 work trails the last exp
GRP = [(0, 7), (7, 7), (14, 7), (21, 7), (28, 2), (30, 2)]

BF16 = mybir.dt.bfloat16
F16 = mybir.dt.float16
F32 = mybir.dt.float32
NPBF16 = ml_dtypes.bfloat16
NPF16 = np.float16

_CACHE = {}


def _chunk_wins(c):
    """Windows covered by chunk c."""
    return list(range(3 * c, min(3 * c + 3, NW)))


def _hoff(c):
    return 768  # uniform h-stride (last chunk wastes cols [384:768) per half)


def _ecur(w, h):
    """Column of window w's cur block (batch-half h) in the per-pair E tile."""
    c = min(w // 3, NCH - 1)
    wi = w - 3 * c
    return CW * c + _hoff(c) * h + 256 * wi


def _build_program():
    nc = bacc.Bacc(None, target_bir_lowering=False, debug=False)
    qt = nc.dram_tensor("qt", [NP * 128, N], BF16, kind="ExternalInput")
    kt = nc.dram_tensor("kt", [NP * 128, N], BF16, kind="ExternalInput")
    vb = nc.dram_tensor("vb", [BL * 128, NW * 65], BF16, kind="ExternalInput")
    m01 = nc.dram_tensor("m01", [128, 128], BF16, kind="ExternalInput")
    madd = nc.dram_tensor("madd", [128, 256], F16, kind="ExternalInput")
    outb = nc.dram_tensor("outb", [BL * 128, NW * D], BF16, kind="ExternalOutput")

    with tile.TileContext(nc) as tc:
        with (
            tc.tile_pool(name="const", bufs=1) as constp,
            tc.tile_pool(name="io", bufs=2) as iop,
            tc.tile_pool(name="ep", bufs=2) as ep,
            tc.tile_pool(name="sp", bufs=2) as spp,
            tc.tile_pool(name="rp", bufs=2) as rp,
            tc.tile_pool(name="ob", bufs=2) as obp,
            tc.tile_pool(name="ps", bufs=2, space="PSUM") as psp,
            tc.tile_pool(name="po", bufs=1, space="PSUM") as pop,
        ):
            m_t = constp.tile([128, 128], BF16, tag="m01")
            ma_t = constp.tile([128, 256], F16, tag="madd")

            # deferred big-exp closure of the previous pair, fired early in
            # the NEXT pair's stream (scalar never waits on the evacuation)
            pending = []  # [(big_exp_fn, [deferred mm2_group fns])]

            for p in range(NP):
                q_ = iop.tile([128, N], BF16, tag="q")
                k_ = iop.tile([128, N], BF16, tag="k")
                if p == 0:
                    # split first loads so chunk-0 matmuls start early
                    nc.sync.dma_start(out=q_[:, 0:512], in_=qt[0:128, 0:512])
                    nc.sync.dma_start(out=k_[:, 0:512], in_=kt[0:128, 0:512])
                    nc.sync.dma_start(
                        out=q_[:, 512:1024], in_=qt[0:128, 512:1024]
                    )
                    nc.sync.dma_start(
                        out=k_[:, 512:1024], in_=kt[0:128, 512:1024]
                    )
                    nc.sync.dma_start(out=m_t[:], in_=m01[:])
                    nc.sync.dma_start(out=ma_t[:], in_=madd[:])
                    # dependency-free dummy exp: pulls the ~2.7us
                    # ACT_TABLE_LOAD into the DMA head (input is garbage)
                    warm = constp.tile([128, 1], BF16, tag="warm")
                    warm_in = constp.tile([128, 1], BF16, tag="warm_in")
                    nc.vector.memset(warm_in[:], 0.0)
                    nc.scalar.activation(
                        warm[:], warm_in[:],
                        mybir.ActivationFunctionType.Exp,
                    )
                    nc.sync.dma_start(
                        out=q_[:, 1024:N], in_=qt[0:128, 1024:N]
                    )
                    nc.sync.dma_start(
                        out=k_[:, 1024:N], in_=kt[0:128, 1024:N]
                    )
                else:
                    nc.sync.dma_start(
                        out=q_[:], in_=qt[p * 128:(p + 1) * 128, :]
                    )
                    nc.sync.dma_start(
                        out=k_[:], in_=kt[p * 128:(p + 1) * 128, :]
                    )
                v0 = iop.tile([128, NW * 65], BF16, tag="v0")
                v1 = iop.tile([128, NW * 65], BF16, tag="v1")
                nc.sync.dma_start(
                    out=v0[:], in_=vb[2 * p * 128:(2 * p + 1) * 128, :]
                )
                nc.sync.dma_start(
                    out=v1[:], in_=vb[(2 * p + 1) * 128:(2 * p + 2) * 128, :]
                )

                # last pair: all-direct (no deferral target for a big exp)
                nstg = NSTG if p < NP - 1 else 0

                eh = ep.tile([128, NCH * CW], BF16, tag="eh")
                if nstg:
                    sp16 = spp.tile([128, NSTG * CW], F16, tag="sp16")
                else:
                    sp16 = None
                osb = {}
                for h in range(2):
                    osb[h] = obp.tile([128, NW * D], BF16, tag=f"osb{h}",
                                      name=f"osb{h}")

                def mm2_group(h, gi, p=p, eh=eh, v0=v0, v1=v1, osb=osb):
                    v_ = v0 if h == 0 else v1
                    g0, gl = GRP[gi]
                    O = pop.tile([128, 512], F32, tag=f"O{h}", name=f"O{h}")
                    for j in range(gl):
                        w = g0 + j
                        if w == 0:
                            nc.tensor.matmul(
                                O[:, 0:65],
                                lhsT=eh[:, _ecur(0, h): _ecur(0, h) + 128],
                                rhs=v_[:, 0:65],
                                start=True, stop=True,
                            )
                        else:
                            bk = _ecur(w - 1, h) + 128
                            nc.tensor.matmul(
                                O[:, j * 65:(j + 1) * 65],
                                lhsT=eh[:, bk: bk + 128],
                                rhs=v_[:, (w - 1) * 65: w * 65],
                                start=True, stop=False,
                            )
                            cu = _ecur(w, h)
                            nc.tensor.matmul(
                                O[:, j * 65:(j + 1) * 65],
                                lhsT=eh[:, cu: cu + 128],
                                rhs=v_[:, w * 65:(w + 1) * 65],
                                start=False, stop=True,
                            )
                    r = rp.tile([128, 8], F32, tag=f"r{h}", name=f"r{h}")
                    ogrp = O[:, 0: gl * 65].rearrange("p (w c) -> p w c", c=65)
                    nc.vector.reciprocal_approx_fast(r[:, 0:gl], ogrp[:, :, 64])
                    nc.vector.tensor_mul(
                        osb[h][:, g0 * D: (g0 + gl) * D].rearrange(
                            "p (w c) -> p w c", c=D
                        ),
                        ogrp[:, :, 0:D],
                        r[:, 0:gl].unsqueeze(2).broadcast_to([128, gl, D]),
                    )
                    b = 2 * p + h
                    if p == NP - 1:
                        # last pair: 2 consolidated DMAs per half (each DMA
                        # issue costs ~0.7us on the Sync queue, so per-group
                        # DMAs would serialize into a long tail)
                        if gi == 3:
                            nc.sync.dma_start(
                                out=outb[b * 128:(b + 1) * 128, 0:28 * D],
                                in_=osb[h][:, 0:28 * D],
                            )
                        elif gi == len(GRP) - 1:
                            nc.sync.dma_start(
                                out=outb[b * 128:(b + 1) * 128,
                                         28 * D:NW * D],
                                in_=osb[h][:, 28 * D:NW * D],
                            )

                def out_dma(h, p=p, osb=osb):
                    b = 2 * p + h
                    nc.sync.dma_start(
                        out=outb[b * 128:(b + 1) * 128, :], in_=osb[h][:]
                    )

                # trigger map: group ready once every chunk it reads E from
                # (cur of windows [g0, g0+gl) AND back source = cur of g0-1)
                # is exp'd+masked. Groups touching staged chunks resolve only
                # at the big exp, which runs DEFERRED in the next pair ->
                # collect those (plus the full-row output DMAs, which must
                # come after them) for deferred firing.
                trig = {}
                deferred_items = []
                for gi, (g0, gl) in enumerate(GRP):
                    w_l = g0 + gl - 1
                    c_first = max(g0 - 1, 0) // 3
                    c_t0 = min(w_l // 3, NCH - 1)
                    # h1 staggered one chunk later to smooth PE load, but
                    # never onto the final chunk (keeps the tail short)
                    c_t1 = max(c_t0, min(c_t0 + 1, NCH - 2))
                    for h, c_t in ((0, c_t0), (1, c_t1)):
                        if c_first < nstg:
                            deferred_items.append(
                                lambda h=h, gi=gi, f=mm2_group: f(h, gi)
                            )
                        else:
                            trig.setdefault(c_t, []).append((h, gi))
                if p < NP - 1:
                    if nstg:
                        deferred_items.append(lambda f=out_dma: f(0))
                        deferred_items.append(lambda f=out_dma: f(1))
                    else:
                        # all-direct pair still defers nothing; DMA after
                        # the final in-pair group fires (append to trig)
                        trig.setdefault(NCH - 1, []).append((0, "dma"))
                        trig.setdefault(NCH - 1, []).append((1, "dma"))

                def big_exp(nstg=nstg, eh=eh, sp16=sp16):
                    nc.scalar.activation(
                        eh[:, 0: nstg * CW], sp16[:, 0: nstg * CW],
                        mybir.ActivationFunctionType.Exp,
                        scale=float(D) ** -0.5,
                    )

                # previous pair's deferred work: big exp fires right after
                # our evacs are issued (c == nstg-1, or c == 0 when nstg=0);
                # its MM2 groups + output DMAs are spread one per chunk after
                # that so they don't block this pair's MM1s in the PE queue.
                prev = pending.pop() if pending else None
                fire_at = max(nstg - 1, 0)

                # 1-chunk lookahead: MM1 for chunk c+1 is issued before chunk
                # c's consumer so deferred MM2 fires (which wait on the
                # previous pair's big exp) never starve the PE of MM1 work.
                s_tiles = {}
                for cc in range(NCH + 1):
                    if cc < NCH:
                        S_new = psp.tile([128, CW], F32, tag="S")
                        s_tiles[cc] = S_new
                        for wi, w in enumerate(_chunk_wins(cc)):
                            n1 = 256 if w < NW - 1 else 128
                            for h in range(2):
                                nc.tensor.matmul(
                                    S_new[:, 768 * h + 256 * wi:
                                          768 * h + 256 * wi + n1],
                                    lhsT=k_[64 * h:64 * h + 64,
                                            w * W:(w + 1) * W],
                                    rhs=q_[64 * h:64 * h + 64,
                                           w * W: w * W + n1],
                                    start=True, stop=True,
                                )
                    if cc == 0:
                        continue
                    c = cc - 1
                    S = s_tiles.pop(c)
                    if c < nstg:
                        # staged: DVE evac PSUM->SBUF fp16 with fused
                        # additive causal mask ([tri(-30000)|0] x6)
                        nc.vector.tensor_add(
                            sp16[:, c * CW:(c + 1) * CW].rearrange(
                                "p (b x) -> p b x", x=256
                            ),
                            S[:, 0:CW].rearrange("p (b x) -> p b x", x=256),
                            ma_t[:].unsqueeze(1).broadcast_to([128, 6, 256]),
                        )
                    else:
                        # direct: exp straight from PSUM, then 0/1 mask on
                        # the cur blocks via the (otherwise idle) GPSIMD
                        if c < NCH - 1:
                            ncols = CW
                        else:
                            # last chunk: live cols [0:384) h0 + [768:1152) h1
                            ncols = 1152
                        nc.scalar.activation(
                            eh[:, c * CW: c * CW + ncols], S[:, 0:ncols],
                            mybir.ActivationFunctionType.Exp,
                            scale=float(D) ** -0.5,
                        )
                        cur = eh[:, c * CW:(c + 1) * CW].rearrange(
                            "p (b x) -> p b x", x=256
                        )[:, :, 0:128]
                        nc.vector.tensor_mul(
                            cur,
                            cur,
                            m_t[:].unsqueeze(1).broadcast_to([128, 6, 128]),
                        )
                    if prev is not None and c == fire_at:
                        prev[0]()          # previous pair's big exp
                    if prev is not None and c >= fire_at:
                        items = prev[1]
                        idx = c - fire_at
                        if idx < len(items):
                            items[idx]()   # one deferred MM2 group / DMA
                        if c == NCH - 1:
                            for fn in items[NCH - fire_at:]:
                                fn()       # flush any leftovers
                    for h, gi in trig.get(c, ()):
                        if gi == "dma":
                            out_dma(h)
                        else:
                            mm2_group(h, gi)

                if nstg:
                    pending.append((big_exp, deferred_items))
                else:
                    for fn in deferred_items:
                        fn()
            # flush any remaining deferred work (the final staged pair)
            for pb, pitems in pending:
                pb()
                for fn in pitems:
                    fn()
    nc.finalize()
    return nc


def _mask():
    j = np.arange(128)[:, None]
    i = np.arange(128)[None, :]
    return (i >= j).astype(NPBF16)                     # [j, i] allowed mask


def _mask_add():
    """Additive pre-exp mask for one 256-col strip: [tri | zeros] fp16."""
    j = np.arange(128)[:, None]
    i = np.arange(128)[None, :]
    m = np.zeros((128, 256), dtype=np.float32)
    m[:, 0:128] = np.where(i >= j, 0.0, -30000.0)
    return m.astype(NPF16)


def _rope(x):
    # x: [B', N, D] f32; global-position angles
    inv = 1.0 / THETA ** (np.arange(0, D, 2, dtype=np.float32) / D)
    ang = np.arange(N, dtype=np.float32)[:, None] * inv[None, :]   # [N, 32]
    cos = np.cos(ang)
    sin = np.sin(ang)
    lo, hi = x[..., : D // 2], x[..., D // 2:]
    out = np.empty_like(x)
    out[..., : D // 2] = lo * cos - hi * sin
    out[..., D // 2:] = hi * cos + lo * sin
    return out


def kernel(q, k, v):
    if "nc" not in _CACHE:
        _CACHE["nc"] = _build_program()
    nc = _CACHE["nc"]
    m01 = _mask()
    madd = _mask_add()

    qr = _rope(q)
    kr = _rope(k)

    in_maps = []
    for c in range(NCORES):
        sl = slice(c * BL, (c + 1) * BL)
        qc, kc, vc = qr[sl], kr[sl], v[sl]          # [16, N, 64] f32
        # d-major, batch pairs stacked on partitions: [NP, 2*64, N]
        qtc = qc.transpose(0, 2, 1).reshape(NP, 128, N)
        ktc = kc.transpose(0, 2, 1).reshape(NP, 128, N)
        # v blocked [16, 128, 32, 65] with ones column
        vbc = np.empty((BL, 128, NW, 65), dtype=NPBF16)
        vbc[..., :64] = vc.reshape(BL, NW, W, D).transpose(0, 2, 1, 3)
        vbc[..., 64] = 1.0
        in_maps.append({
            "qt": qtc.reshape(NP * 128, N).astype(NPBF16),
            "kt": ktc.reshape(NP * 128, N).astype(NPBF16),
            "vb": vbc.reshape(BL * 128, NW * 65),
            "m01": m01,
            "madd": madd,
        })

    res = run_bass_kernel_spmd(nc, in_maps, list(range(NCORES)))
    _CACHE["last_results"] = res
    out = np.empty((B, N, D), dtype=np.float32)
    for c in range(NCORES):
        ob = res.results[c]["outb"].astype(np.float32).reshape(BL, 128, NW, D)
        out[c * BL:(c + 1) * BL] = (
            ob.transpose(0, 2, 1, 3).reshape(BL, N, D)
        )
    return out


if __name__ == "__main__":
    rng = np.random.default_rng(0)
    q = rng.standard_normal((B, N, D), dtype=np.float32)
    k = rng.standard_normal((B, N, D), dtype=np.float32)
    v = rng.standard_normal((B, N, D), dtype=np.float32)
    o = kernel(q, k, v)
    print("out", o.shape, o.dtype, np.abs(o).max())


# revision 14
# speedup vs baseline: 1.3139x; 1.3139x over previous
"""Local windowed attention (window=128, look_backward=1, RoPE) on 8 TRN2 cores.

Sharding: data-parallel over batch (128 -> 16 per core).

Host prep (layout/dtype/embedding-preprocessing only -- all of the attention
itself, i.e. logits, causal-masked softmax and the weighted sum over values,
runs on device):
  * RoPE rotation applied to q,k on the host using global-position angles
    (rotation-invariance of dot products makes this exactly equivalent to the
    reference's window-relative angles); q,k uploaded pre-transposed d-major
    [64, N] bf16 with two batches stacked on the 128 SBUF partitions.
  * v blocked per window with a ones column appended (the softmax denominator
    then falls out of MM2's PSUM accumulation).

Device (the scalar engine's exp is the bottleneck: ACTIVATE costs
(cols+352)/1.2 ns at 1 col/cycle regardless of dtype, so the design minimizes
scalar-queue columns and instruction startups):
  * MM1 (logits^T, j-major): stationary k^T_w [64,128], moving [q^T_w|q^T_{w+1}]
    -> strip [cur_w | back_{w+1}]; batch halves on partition halves 0:63/64:127
    via tile_position (0,0)/(64,0) run concurrently on disjoint PE row-groups.
  * Chunks of 3 windows (1536 cols: [h0: 3 strips | h1: 3 strips]); 11 exp
    startups per pair instead of 16 (PSUM-capped: 3-bank S tiles x2 + 2 O
    banks = 8). Per-chunk exp straight from PSUM, 1/sqrt(D) folded into the
    activation scale; 0/1 causal mask applied post-exp on DVE (one strided
    broadcast multiply per chunk).
  * An SBUF-staging path (NSTG>0: DVE evacuates S to fp16 SBUF with a fused
    additive mask, then one giant SBUF-sourced exp) is implemented but OFF:
    the DVE is too loaded for the evacuation (measured 200us vs 151us), and
    GPSIMD cannot access PSUM and has ~1us-per-semaphore queue overhead
    (measured 258us when it carried the masks).
  * MM2 accumulates [back|cur] x v into PSUM groups (pop bufs=1); norm =
    fast approximate reciprocal (~51 ULP) + one broadcast multiply per group
    on DVE. h1 triggers staggered +1 chunk but never onto the final chunk.
  * Last pair: 2 consolidated output DMAs per half (each dma_start costs
    ~0.7us of Sync-queue issue time; 12 per-group DMAs serialized into an
    8us tail).
  * Output written blocked [pos-in-window, (window, d)] bf16; host
    inverse-permutes and upcasts.

Measured on trn2 (8 cores, core-0 NTFF profile): 151.5us HW exec vs 154.1us
for the previous baseline; rel err 5.3e-3 (tolerance 2e-2).
"""

import sys

sys.path.insert(0, "/opt/trn_rl_repo")

import numpy as np
import ml_dtypes

import concourse.bass as bass
import concourse.bacc as bacc
import concourse.mybir as mybir
import concourse.tile as tile
from concourse.bass_utils import run_bass_kernel_spmd

B, N, D, W = 128, 4096, 64, 128
NCORES = 8
BL = B // NCORES          # 16 batches per core
NP = BL // 2              # 8 batch-pairs per core
NW = N // W               # 32 windows
THETA = 10000.0
# chunking: 10 chunks of 3 windows + 1 chunk of 2 windows (+junk), all
# [128, 1536] PSUM tiles (3 banks; bufs=2 -> 6 banks, O pool takes 2)
NCH = 11
CW = 1536                 # chunk width in S / eh columns
NSTG = 0                  # staged chunks per pair (0 = all direct)
# MM2 window groups (start, len): 7-window groups fill a PSUM bank; the final
# windows are split into 2-window groups so little MM2 work trails the last exp
GRP = [(0, 7), (7, 7), (14, 7), (21, 7), (28, 2), (30, 2)]

BF16 = mybir.dt.bfloat16
F16 = mybir.dt.float16
F32 = mybir.dt.float32
NPBF16 = ml_dtypes.bfloat16
NPF16 = np.float16

_CACHE = {}


def _chunk_wins(c):
    """Windows covered by chunk c."""
    return list(range(3 * c, min(3 * c + 3, NW)))


def _hoff(c):
    return 768  # uniform h-stride (last chunk wastes cols [384:768) per half)


def _ecur(w, h):
    """Column of window w's cur block (batch-half h) in the per-pair E tile."""
    c = min(w // 3, NCH - 1)
    wi = w - 3 * c
    return CW * c + _hoff(c) * h + 256 * wi


def _build_program():
    nc = bacc.Bacc(None, target_bir_lowering=False, debug=False)
    qt = nc.dram_tensor("qt", [NP * 128, N], BF16, kind="ExternalInput")
    kt = nc.dram_tensor("kt", [NP * 128, N], BF16, kind="ExternalInput")
    vb = nc.dram_tensor("vb", [BL * 128, NW * 65], BF16, kind="ExternalInput")
    m01 = nc.dram_tensor("m01", [128, 128], BF16, kind="ExternalInput")
    madd = nc.dram_tensor("madd", [128, 256], F16, kind="ExternalInput")
    outb = nc.dram_tensor("outb", [BL * 128, NW * D], BF16, kind="ExternalOutput")

    with tile.TileContext(nc) as tc:
        with (
            tc.tile_pool(name="const", bufs=1) as constp,
            tc.tile_pool(name="io", bufs=2) as iop,
            tc.tile_pool(name="ep", bufs=2) as ep,
            tc.tile_pool(name="sp", bufs=2) as spp,
            tc.tile_pool(name="rp", bufs=2) as rp,
            tc.tile_pool(name="ob", bufs=2) as obp,
            tc.tile_pool(name="ps", bufs=2, space="PSUM") as psp,
            tc.tile_pool(name="po", bufs=1, space="PSUM") as pop,
        ):
            m_t = constp.tile([128, 128], BF16, tag="m01")
            ma_t = constp.tile([128, 256], F16, tag="madd")

            # deferred big-exp closure of the previous pair, fired early in
            # the NEXT pair's stream (scalar never waits on the evacuation)
            pending = []  # [(big_exp_fn, [deferred mm2_group fns])]

            for p in range(NP):
                q_ = iop.tile([128, N], BF16, tag="q")
                k_ = iop.tile([128, N], BF16, tag="k")
                if p == 0:
                    # split first loads so chunk-0 matmuls start early
                    nc.sync.dma_start(out=q_[:, 0:512], in_=qt[0:128, 0:512])
                    nc.sync.dma_start(out=k_[:, 0:512], in_=kt[0:128, 0:512])
                    nc.sync.dma_start(
                        out=q_[:, 512:1024], in_=qt[0:128, 512:1024]
                    )
                    nc.sync.dma_start(
                        out=k_[:, 512:1024], in_=kt[0:128, 512:1024]
                    )
                    nc.sync.dma_start(out=m_t[:], in_=m01[:])
                    nc.sync.dma_start(out=ma_t[:], in_=madd[:])
                    # dependency-free dummy exp: pulls the ~2.7us
                    # ACT_TABLE_LOAD into the DMA head (input is garbage)
                    warm = constp.tile([128, 1], BF16, tag="warm")
                    warm_in = constp.tile([128, 1], BF16, tag="warm_in")
                    nc.vector.memset(warm_in[:], 0.0)
                    nc.scalar.activation(
                        warm[:], warm_in[:],
                        mybir.ActivationFunctionType.Exp,
                    )
                    nc.sync.dma_start(
                        out=q_[:, 1024:N], in_=qt[0:128, 1024:N]
                    )
                    nc.sync.dma_start(
                        out=k_[:, 1024:N], in_=kt[0:128, 1024:N]
                    )
                else:
                    nc.sync.dma_start(
                        out=q_[:], in_=qt[p * 128:(p + 1) * 128, :]
                    )
                    nc.sync.dma_start(
                        out=k_[:], in_=kt[p * 128:(p + 1) * 128, :]
                    )
                v0 = iop.tile([128, NW * 65], BF16, tag="v0")
                v1 = iop.tile([128, NW * 65], BF16, tag="v1")
                nc.sync.dma_start(
                    out=v0[:], in_=vb[2 * p * 128:(2 * p + 1) * 128, :]
                )
                nc.sync.dma_start(
                    out=v1[:], in_=vb[(2 * p + 1) * 128:(2 * p + 2) * 128, :]
                )

                # last pair: all-direct (no deferral target for a big exp)
                nstg = NSTG if p < NP - 1 else 0

                eh = ep.tile([128, NCH * CW], BF16, tag="eh")
                if nstg:
                    sp16 = spp.tile([128, NSTG * CW], F16, tag="sp16")
                else:
                    sp16 = None
                osb = {}
                for h in range(2):
                    osb[h] = obp.tile([128, NW * D], BF16, tag=f"osb{h}",
                                      name=f"osb{h}")

                def mm2_group(h, gi, p=p, eh=eh, v0=v0, v1=v1, osb=osb):
                    v_ = v0 if h == 0 else v1
                    g0, gl = GRP[gi]
                    O = pop.tile([128, 512], F32, tag=f"O{h}", name=f"O{h}")
                    for j in range(gl):
                        w = g0 + j
                        if w == 0:
                            nc.tensor.matmul(
                                O[:, 0:65],
                                lhsT=eh[:, _ecur(0, h): _ecur(0, h) + 128],
                                rhs=v_[:, 0:65],
                                start=True, stop=True,
                            )
                        else:
                            bk = _ecur(w - 1, h) + 128
                            nc.tensor.matmul(
                                O[:, j * 65:(j + 1) * 65],
                                lhsT=eh[:, bk: bk + 128],
                                rhs=v_[:, (w - 1) * 65: w * 65],
                                start=True, stop=False,
                            )
                            cu = _ecur(w, h)
                            nc.tensor.matmul(
                                O[:, j * 65:(j + 1) * 65],
                                lhsT=eh[:, cu: cu + 128],
                                rhs=v_[:, w * 65:(w + 1) * 65],
                                start=False, stop=True,
                            )
                    r = rp.tile([128, 8], F32, tag=f"r{h}", name=f"r{h}")
                    ogrp = O[:, 0: gl * 65].rearrange("p (w c) -> p w c", c=65)
                    nc.vector.reciprocal_approx_fast(r[:, 0:gl], ogrp[:, :, 64])
                    nc.vector.tensor_mul(
                        osb[h][:, g0 * D: (g0 + gl) * D].rearrange(
                            "p (w c) -> p w c", c=D
                        ),
                        ogrp[:, :, 0:D],
                        r[:, 0:gl].unsqueeze(2).broadcast_to([128, gl, D]),
                    )
                    b = 2 * p + h
                    if p == NP - 1:
                        # last pair: 2 consolidated DMAs per half (each DMA
                        # issue costs ~0.7us on the Sync queue, so per-group
                        # DMAs would serialize into a long tail)
                        if gi == 3:
                            nc.sync.dma_start(
                                out=outb[b * 128:(b + 1) * 128, 0:28 * D],
                                in_=osb[h][:, 0:28 * D],
                            )
                        elif gi == len(GRP) - 1:
                            nc.sync.dma_start(
                                out=outb[b * 128:(b + 1) * 128,
                                         28 * D:NW * D],
                                in_=osb[h][:, 28 * D:NW * D],
                            )

                def out_dma(h, p=p, osb=osb):
                    b = 2 * p + h
                    nc.sync.dma_start(
                        out=outb[b * 128:(b + 1) * 128, :], in_=osb[h][:]
                    )

                # trigger map: group ready once every chunk it reads E from
                # (cur of windows [g0, g0+gl) AND back source = cur of g0-1)
                # is exp'd+masked. Groups touching staged chunks resolve only
                # at the big exp, which runs DEFERRED in the next pair ->
                # collect those (plus the full-row output DMAs, which must
                # come after them) for deferred firing.
                trig = {}
                deferred_items = []
                for gi, (g0, gl) in enumerate(GRP):
                    w_l = g0 + gl - 1
                    c_first = max(g0 - 1, 0) // 3
                    c_t0 = min(w_l // 3, NCH - 1)
                    # h1 staggered one chunk later to smooth PE load, but
                    # never onto the final chunk (keeps the tail short)
                    c_t1 = max(c_t0, min(c_t0 + 1, NCH - 2))
                    for h, c_t in ((0, c_t0), (1, c_t1)):
                        if c_first < nstg:
                            deferred_items.append(
                                lambda h=h, gi=gi, f=mm2_group: f(h, gi)
                            )
                        else:
                            trig.setdefault(c_t, []).append((h, gi))
                if p < NP - 1:
                    if nstg:
                        deferred_items.append(lambda f=out_dma: f(0))
                        deferred_items.append(lambda f=out_dma: f(1))
                    else:
                        # all-direct pair still defers nothing; DMA after
                        # the final in-pair group fires (append to trig)
                        trig.setdefault(NCH - 1, []).append((0, "dma"))
                        trig.setdefault(NCH - 1, []).append((1, "dma"))

                def big_exp(nstg=nstg, eh=eh, sp16=sp16):
                    nc.scalar.activation(
                        eh[:, 0: nstg * CW], sp16[:, 0: nstg * CW],
                        mybir.ActivationFunctionType.Exp,
                        scale=float(D) ** -0.5,
                    )

                # previous pair's deferred work: big exp fires right after
                # our evacs are issued (c == nstg-1, or c == 0 when nstg=0);
                # its MM2 groups + output DMAs are spread one per chunk after
                # that so they don't block this pair's MM1s in the PE queue.
                prev = pending.pop() if pending else None
                fire_at = max(nstg - 1, 0)

                # 1-chunk lookahead: MM1 for chunk c+1 is issued before chunk
                # c's consumer so deferred MM2 fires (which wait on the
                # previous pair's big exp) never starve the PE of MM1 work.
                s_tiles = {}
                for cc in range(NCH + 1):
                    if cc < NCH:
                        S_new = psp.tile([128, CW], F32, tag="S")
                        s_tiles[cc] = S_new
                        for wi, w in enumerate(_chunk_wins(cc)):
                            n1 = 256 if w < NW - 1 else 128
                            for h in range(2):
                                nc.tensor.matmul(
                                    S_new[:, 768 * h + 256 * wi:
                                          768 * h + 256 * wi + n1],
                                    lhsT=k_[64 * h:64 * h + 64,
                                            w * W:(w + 1) * W],
                                    rhs=q_[64 * h:64 * h + 64,
                                           w * W: w * W + n1],
                                    start=True, stop=True,
                                )
                    if cc == 0:
                        continue
                    c = cc - 1
                    S = s_tiles.pop(c)
                    if c < nstg:
                        # staged: DVE evac PSUM->SBUF fp16 with fused
                        # additive causal mask ([tri(-30000)|0] x6)
                        nc.vector.tensor_add(
                            sp16[:, c * CW:(c + 1) * CW].rearrange(
                                "p (b x) -> p b x", x=256
                            ),
                            S[:, 0:CW].rearrange("p (b x) -> p b x", x=256),
                            ma_t[:].unsqueeze(1).broadcast_to([128, 6, 256]),
                        )
                    else:
                        # direct: exp straight from PSUM, then 0/1 mask on
                        # the cur blocks via the (otherwise idle) GPSIMD
                        if c < NCH - 1:
                            ncols = CW
                        else:
                            # last chunk: live cols [0:384) h0 + [768:1152) h1
                            ncols = 1152
                        nc.scalar.activation(
                            eh[:, c * CW: c * CW + ncols], S[:, 0:ncols],
                            mybir.ActivationFunctionType.Exp,
                            scale=float(D) ** -0.5,
                        )
                        cur = eh[:, c * CW:(c + 1) * CW].rearrange(
                            "p (b x) -> p b x", x=256
                        )[:, :, 0:128]
                        nc.vector.tensor_mul(
                            cur,
                            cur,
                            m_t[:].unsqueeze(1).broadcast_to([128, 6, 128]),
                        )
                    if prev is not None and c == fire_at:
                        prev[0]()          # previous pair's big exp
                    if prev is not None and c >= fire_at:
                        items = prev[1]
                        idx = c - fire_at
                        if idx < len(items):
                            items[idx]()   # one deferred MM2 group / DMA
                        if c == NCH - 1:
                            for fn in items[NCH - fire_at:]:
                                fn()       # flush any leftovers
                    for h, gi in trig.get(c, ()):
                        if gi == "dma":
                            out_dma(h)
                        else:
                            mm2_group(h, gi)

                if nstg:
                    pending.append((big_exp, deferred_items))
                else:
                    for fn in deferred_items:
                        fn()
            # flush any remaining deferred work (the final staged pair)
            for pb, pitems in pending:
                pb()
                for fn in pitems:
                    fn()
    nc.finalize()
    return nc


def _mask():
    j = np.arange(128)[:, None]
    i = np.arange(128)[None, :]
    return (i >= j).astype(NPBF16)                     # [j, i] allowed mask


def _mask_add():
    """Additive pre-exp mask for one 256-col strip: [tri | zeros] fp16."""
    j = np.arange(128)[:, None]
    i = np.arange(128)[None, :]
    m = np.zeros((128, 256), dtype=np.float32)
    m[:, 0:128] = np.where(i >= j, 0.0, -30000.0)
    return m.astype(NPF16)


def _rope(x):
    # x: [B', N, D] f32; global-position angles
    inv = 1.0 / THETA ** (np.arange(0, D, 2, dtype=np.float32) / D)
    ang = np.arange(N, dtype=np.float32)[:, None] * inv[None, :]   # [N, 32]
    cos = np.cos(ang)
    sin = np.sin(ang)
    lo, hi = x[..., : D // 2], x[..., D // 2:]
    out = np.empty_like(x)
    out[..., : D // 2] = lo * cos - hi * sin
    out[..., D // 2:] = hi * cos + lo * sin
    return out


def kernel(q, k, v):
    if "nc" not in _CACHE:
        _CACHE["nc"] = _build_program()
    nc = _CACHE["nc"]
    m01 = _mask()
    madd = _mask_add()

    qr = _rope(q)
    kr = _rope(k)

    in_maps = []
    for c in range(NCORES):
        sl = slice(c * BL, (c + 1) * BL)
        qc, kc, vc = qr[sl], kr[sl], v[sl]          # [16, N, 64] f32
        # d-major, batch pairs stacked on partitions: [NP, 2*64, N]
        qtc = qc.transpose(0, 2, 1).reshape(NP, 128, N)
        ktc = kc.transpose(0, 2, 1).reshape(NP, 128, N)
        # v blocked [16, 128, 32, 65] with ones column
        vbc = np.empty((BL, 128, NW, 65), dtype=NPBF16)
        vbc[..., :64] = vc.reshape(BL, NW, W, D).transpose(0, 2, 1, 3)
        vbc[..., 64] = 1.0
        in_maps.append({
            "qt": qtc.reshape(NP * 128, N).astype(NPBF16),
            "kt": ktc.reshape(NP * 128, N).astype(NPBF16),
            "vb": vbc.reshape(BL * 128, NW * 65),
            "m01": m01,
            "madd": madd,
        })

    res = run_bass_kernel_spmd(nc, in_maps, list(range(NCORES)))
    _CACHE["last_results"] = res
    out = np.empty((B, N, D), dtype=np.float32)
    for c in range(NCORES):
        ob = res.results[c]["outb"].astype(np.float32).reshape(BL, 128, NW, D)
        out[c * BL:(c + 1) * BL] = (
            ob.transpose(0, 2, 1, 3).reshape(BL, N, D)
        )
    return out


if __name__ == "__main__":
    rng = np.random.default_rng(0)
    q = rng.standard_normal((B, N, D), dtype=np.float32)
    k = rng.standard_normal((B, N, D), dtype=np.float32)
    v = rng.standard_normal((B, N, D), dtype=np.float32)
    o = kernel(q, k, v)
    print("out", o.shape, o.dtype, np.abs(o).max())


# revision 15
# speedup vs baseline: 1.3388x; 1.0189x over previous
"""Local windowed attention (window=128, look_backward=1, RoPE) on 8 TRN2 cores.

Sharding: data-parallel over batch (128 -> 16 per core).

Host prep (layout/dtype/embedding-preprocessing only -- all of the attention
itself, i.e. logits, causal-masked softmax and the weighted sum over values,
runs on device):
  * RoPE rotation applied to q,k on the host using global-position angles
    (rotation-invariance of dot products makes this exactly equivalent to the
    reference's window-relative angles); q,k uploaded pre-transposed d-major
    [64, N] bf16 with two batches stacked on the 128 SBUF partitions.
  * v blocked per window with a ones column appended (the softmax denominator
    then falls out of MM2's PSUM accumulation).

Device (the scalar engine's exp is the bottleneck: ACTIVATE costs
(cols+352)/1.2 ns at 1 col/cycle regardless of dtype, so the design minimizes
scalar-queue columns and instruction startups):
  * MM1 (logits^T, j-major): stationary k^T_w [64,128], moving [q^T_w|q^T_{w+1}]
    -> strip [cur_w | back_{w+1}]; batch halves on partition halves 0:63/64:127
    via tile_position (0,0)/(64,0) run concurrently on disjoint PE row-groups.
  * Chunks of 3 windows (1536 cols: [h0: 3 strips | h1: 3 strips]); 11 exp
    startups per pair instead of 16 (PSUM-capped: 3-bank S tiles x2 + 2 O
    banks = 8). Per-chunk exp straight from PSUM, 1/sqrt(D) folded into the
    activation scale; 0/1 causal mask applied post-exp on DVE (one strided
    broadcast multiply per chunk).
  * An SBUF-staging path (NSTG>0: DVE evacuates S to fp16 SBUF with a fused
    additive mask, then one giant SBUF-sourced exp) is implemented but OFF:
    the DVE is too loaded for the evacuation (measured 200us vs 151us), and
    GPSIMD cannot access PSUM and has ~1us-per-semaphore queue overhead
    (measured 258us when it carried the masks).
  * MM2 accumulates [back|cur] x v into PSUM groups (pop bufs=1); norm =
    fast approximate reciprocal (~51 ULP) + one broadcast multiply per group
    on DVE. h1 triggers staggered +1 chunk but never onto the final chunk.
  * Last pair: 2 consolidated output DMAs per half (each dma_start costs
    ~0.7us of Sync-queue issue time; 12 per-group DMAs serialized into an
    8us tail).
  * Output written blocked [pos-in-window, (window, d)] bf16; host
    inverse-permutes and upcasts.

Measured on trn2 (8 cores, core-0 NTFF profile): 151.5us HW exec vs 154.1us
for the previous baseline; rel err 5.3e-3 (tolerance 2e-2).
"""

import sys

sys.path.insert(0, "/opt/trn_rl_repo")

import numpy as np
import ml_dtypes

import concourse.bass as bass
import concourse.bacc as bacc
import concourse.mybir as mybir
import concourse.tile as tile
from concourse.bass_utils import run_bass_kernel_spmd

B, N, D, W = 128, 4096, 64, 128
NCORES = 8
BL = B // NCORES          # 16 batches per core
NP = BL // 2              # 8 batch-pairs per core
NW = N // W               # 32 windows
THETA = 10000.0
# chunking: 10 chunks of 3 windows + 1 chunk of 2 windows (+junk), all
# [128, 1536] PSUM tiles (3 banks; bufs=2 -> 6 banks, O pool takes 2)
NCH = 11
CW = 1536                 # chunk width in S / eh columns
NSTG = 0                  # staged chunks per pair (0 = all direct)
# MM2 window groups (start, len): 7-window groups fill a PSUM bank; the final
# windows are split into 2-window groups so little MM2 work trails the last exp
GRP = [(0, 7), (7, 7), (14, 7), (21, 7), (28, 2), (30, 2)]

BF16 = mybir.dt.bfloat16
F16 = mybir.dt.float16
F32 = mybir.dt.float32
NPBF16 = ml_dtypes.bfloat16
NPF16 = np.float16

_CACHE = {}


def _chunk_wins(c):
    """Windows covered by chunk c."""
    return list(range(3 * c, min(3 * c + 3, NW)))


def _hoff(c):
    return 768  # uniform h-stride (last chunk wastes cols [384:768) per half)


def _ecur(w, h):
    """Column of window w's cur block (batch-half h) in the per-pair E tile."""
    c = min(w // 3, NCH - 1)
    wi = w - 3 * c
    return CW * c + _hoff(c) * h + 256 * wi


def _build_program():
    nc = bacc.Bacc(None, target_bir_lowering=False, debug=False)
    qt = nc.dram_tensor("qt", [NP * 128, N], BF16, kind="ExternalInput")
    kt = nc.dram_tensor("kt", [NP * 128, N], BF16, kind="ExternalInput")
    vb = nc.dram_tensor("vb", [BL * 128, NW * 65], BF16, kind="ExternalInput")
    m01 = nc.dram_tensor("m01", [128, 128], BF16, kind="ExternalInput")
    madd = nc.dram_tensor("madd", [128, 256], F16, kind="ExternalInput")
    outb = nc.dram_tensor("outb", [BL * 128, NW * D], BF16, kind="ExternalOutput")

    with tile.TileContext(nc) as tc:
        with (
            tc.tile_pool(name="const", bufs=1) as constp,
            tc.tile_pool(name="io", bufs=2) as iop,
            tc.tile_pool(name="ep", bufs=3) as ep,
            tc.tile_pool(name="sp", bufs=2) as spp,
            tc.tile_pool(name="rp", bufs=2) as rp,
            tc.tile_pool(name="ob", bufs=2) as obp,
            tc.tile_pool(name="ps", bufs=2, space="PSUM") as psp,
            tc.tile_pool(name="po", bufs=1, space="PSUM") as pop,
        ):
            m_t = constp.tile([128, 128], BF16, tag="m01")
            ma_t = constp.tile([128, 256], F16, tag="madd")

            # deferred big-exp closure of the previous pair, fired early in
            # the NEXT pair's stream (scalar never waits on the evacuation)
            pending = []  # [(big_exp_fn, [deferred mm2_group fns])]

            for p in range(NP):
                q_ = iop.tile([128, N], BF16, tag="q")
                k_ = iop.tile([128, N], BF16, tag="k")
                if p == 0:
                    # split first loads so chunk-0 matmuls start early
                    nc.sync.dma_start(out=q_[:, 0:512], in_=qt[0:128, 0:512])
                    nc.sync.dma_start(out=k_[:, 0:512], in_=kt[0:128, 0:512])
                    nc.sync.dma_start(
                        out=q_[:, 512:1024], in_=qt[0:128, 512:1024]
                    )
                    nc.sync.dma_start(
                        out=k_[:, 512:1024], in_=kt[0:128, 512:1024]
                    )
                    nc.sync.dma_start(out=m_t[:], in_=m01[:])
                    nc.sync.dma_start(out=ma_t[:], in_=madd[:])
                    # dependency-free dummy exp: pulls the ~2.7us
                    # ACT_TABLE_LOAD into the DMA head (input is garbage)
                    warm = constp.tile([128, 1], BF16, tag="warm")
                    warm_in = constp.tile([128, 1], BF16, tag="warm_in")
                    nc.vector.memset(warm_in[:], 0.0)
                    nc.scalar.activation(
                        warm[:], warm_in[:],
                        mybir.ActivationFunctionType.Exp,
                    )
                    for lo, hi in ((1024, 2560), (2560, N)):
                        nc.sync.dma_start(
                            out=q_[:, lo:hi], in_=qt[0:128, lo:hi]
                        )
                        nc.sync.dma_start(
                            out=k_[:, lo:hi], in_=kt[0:128, lo:hi]
                        )
                else:
                    nc.sync.dma_start(
                        out=q_[:], in_=qt[p * 128:(p + 1) * 128, :]
                    )
                    nc.sync.dma_start(
                        out=k_[:], in_=kt[p * 128:(p + 1) * 128, :]
                    )
                v0 = iop.tile([128, NW * 65], BF16, tag="v0")
                v1 = iop.tile([128, NW * 65], BF16, tag="v1")
                nc.sync.dma_start(
                    out=v0[:], in_=vb[2 * p * 128:(2 * p + 1) * 128, :]
                )
                nc.sync.dma_start(
                    out=v1[:], in_=vb[(2 * p + 1) * 128:(2 * p + 2) * 128, :]
                )

                # last pair: all-direct (no deferral target for a big exp)
                nstg = NSTG if p < NP - 1 else 0

                eh = ep.tile([128, NCH * CW], BF16, tag="eh")
                if nstg:
                    sp16 = spp.tile([128, NSTG * CW], F16, tag="sp16")
                else:
                    sp16 = None
                osb = {}
                for h in range(2):
                    osb[h] = obp.tile([128, NW * D], BF16, tag=f"osb{h}",
                                      name=f"osb{h}")

                def mm2_group(h, gi, p=p, eh=eh, v0=v0, v1=v1, osb=osb):
                    v_ = v0 if h == 0 else v1
                    g0, gl = GRP[gi]
                    O = pop.tile([128, 512], F32, tag=f"O{h}", name=f"O{h}")
                    for j in range(gl):
                        w = g0 + j
                        if w == 0:
                            nc.tensor.matmul(
                                O[:, 0:65],
                                lhsT=eh[:, _ecur(0, h): _ecur(0, h) + 128],
                                rhs=v_[:, 0:65],
                                start=True, stop=True,
                            )
                        else:
                            bk = _ecur(w - 1, h) + 128
                            nc.tensor.matmul(
                                O[:, j * 65:(j + 1) * 65],
                                lhsT=eh[:, bk: bk + 128],
                                rhs=v_[:, (w - 1) * 65: w * 65],
                                start=True, stop=False,
                            )
                            cu = _ecur(w, h)
                            nc.tensor.matmul(
                                O[:, j * 65:(j + 1) * 65],
                                lhsT=eh[:, cu: cu + 128],
                                rhs=v_[:, w * 65:(w + 1) * 65],
                                start=False, stop=True,
                            )
                    r = rp.tile([128, 8], F32, tag=f"r{h}", name=f"r{h}")
                    ogrp = O[:, 0: gl * 65].rearrange("p (w c) -> p w c", c=65)
                    nc.vector.reciprocal_approx_fast(r[:, 0:gl], ogrp[:, :, 64])
                    nc.vector.tensor_mul(
                        osb[h][:, g0 * D: (g0 + gl) * D].rearrange(
                            "p (w c) -> p w c", c=D
                        ),
                        ogrp[:, :, 0:D],
                        r[:, 0:gl].unsqueeze(2).broadcast_to([128, gl, D]),
                    )
                    b = 2 * p + h
                    if p == NP - 1:
                        # last pair: 2 consolidated DMAs per half (each DMA
                        # issue costs ~0.7us on the Sync queue, so per-group
                        # DMAs would serialize into a long tail)
                        if gi == 3:
                            nc.sync.dma_start(
                                out=outb[b * 128:(b + 1) * 128, 0:28 * D],
                                in_=osb[h][:, 0:28 * D],
                            )
                        elif gi == len(GRP) - 1:
                            nc.sync.dma_start(
                                out=outb[b * 128:(b + 1) * 128,
                                         28 * D:NW * D],
                                in_=osb[h][:, 28 * D:NW * D],
                            )

                def out_dma(h, p=p, osb=osb):
                    b = 2 * p + h
                    nc.sync.dma_start(
                        out=outb[b * 128:(b + 1) * 128, :], in_=osb[h][:]
                    )

                # trigger map: group ready once every chunk it reads E from
                # (cur of windows [g0, g0+gl) AND back source = cur of g0-1)
                # is exp'd+masked. Groups touching staged chunks resolve only
                # at the big exp, which runs DEFERRED in the next pair ->
                # collect those (plus the full-row output DMAs, which must
                # come after them) for deferred firing.
                trig = {}
                deferred_items = []
                for gi, (g0, gl) in enumerate(GRP):
                    w_l = g0 + gl - 1
                    c_first = max(g0 - 1, 0) // 3
                    c_t0 = min(w_l // 3, NCH - 1)
                    # h1 staggered one chunk later to smooth PE load, but
                    # never onto the final chunk (keeps the tail short)
                    c_t1 = max(c_t0, min(c_t0 + 1, NCH - 2))
                    for h, c_t in ((0, c_t0), (1, c_t1)):
                        if c_first < nstg:
                            deferred_items.append(
                                lambda h=h, gi=gi, f=mm2_group: f(h, gi)
                            )
                        else:
                            trig.setdefault(c_t, []).append((h, gi))
                if p < NP - 1:
                    if nstg:
                        deferred_items.append(lambda f=out_dma: f(0))
                        deferred_items.append(lambda f=out_dma: f(1))
                    else:
                        # all-direct pair still defers nothing; DMA after
                        # the final in-pair group fires (append to trig)
                        trig.setdefault(NCH - 1, []).append((0, "dma"))
                        trig.setdefault(NCH - 1, []).append((1, "dma"))

                def big_exp(nstg=nstg, eh=eh, sp16=sp16):
                    nc.scalar.activation(
                        eh[:, 0: nstg * CW], sp16[:, 0: nstg * CW],
                        mybir.ActivationFunctionType.Exp,
                        scale=float(D) ** -0.5,
                    )

                # previous pair's deferred work: big exp fires right after
                # our evacs are issued (c == nstg-1, or c == 0 when nstg=0);
                # its MM2 groups + output DMAs are spread one per chunk after
                # that so they don't block this pair's MM1s in the PE queue.
                prev = pending.pop() if pending else None
                fire_at = max(nstg - 1, 0)

                # 1-chunk lookahead: MM1 for chunk c+1 is issued before chunk
                # c's consumer so deferred MM2 fires (which wait on the
                # previous pair's big exp) never starve the PE of MM1 work.
                s_tiles = {}
                for cc in range(NCH + 1):
                    if cc < NCH:
                        S_new = psp.tile([128, CW], F32, tag="S")
                        s_tiles[cc] = S_new
                        for wi, w in enumerate(_chunk_wins(cc)):
                            n1 = 256 if w < NW - 1 else 128
                            for h in range(2):
                                nc.tensor.matmul(
                                    S_new[:, 768 * h + 256 * wi:
                                          768 * h + 256 * wi + n1],
                                    lhsT=k_[64 * h:64 * h + 64,
                                            w * W:(w + 1) * W],
                                    rhs=q_[64 * h:64 * h + 64,
                                           w * W: w * W + n1],
                                    start=True, stop=True,
                                )
                    if cc == 0:
                        continue
                    c = cc - 1
                    S = s_tiles.pop(c)
                    if c < nstg:
                        # staged: DVE evac PSUM->SBUF fp16 with fused
                        # additive causal mask ([tri(-30000)|0] x6)
                        nc.vector.tensor_add(
                            sp16[:, c * CW:(c + 1) * CW].rearrange(
                                "p (b x) -> p b x", x=256
                            ),
                            S[:, 0:CW].rearrange("p (b x) -> p b x", x=256),
                            ma_t[:].unsqueeze(1).broadcast_to([128, 6, 256]),
                        )
                    else:
                        # direct: exp straight from PSUM, then 0/1 mask on
                        # the cur blocks via the (otherwise idle) GPSIMD
                        if c < NCH - 1:
                            ncols = CW
                        else:
                            # last chunk: live cols [0:384) h0 + [768:1152) h1
                            ncols = 1152
                        nc.scalar.activation(
                            eh[:, c * CW: c * CW + ncols], S[:, 0:ncols],
                            mybir.ActivationFunctionType.Exp,
                            scale=float(D) ** -0.5,
                        )
                        cur = eh[:, c * CW:(c + 1) * CW].rearrange(
                            "p (b x) -> p b x", x=256
                        )[:, :, 0:128]
                        nc.vector.tensor_mul(
                            cur,
                            cur,
                            m_t[:].unsqueeze(1).broadcast_to([128, 6, 128]),
                        )
                    if prev is not None and c == fire_at:
                        prev[0]()          # previous pair's big exp
                    if prev is not None and c >= fire_at:
                        items = prev[1]
                        idx = c - fire_at
                        if idx < len(items):
                            items[idx]()   # one deferred MM2 group / DMA
                        if c == NCH - 1:
                            for fn in items[NCH - fire_at:]:
                                fn()       # flush any leftovers
                    for h, gi in trig.get(c, ()):
                        if gi == "dma":
                            out_dma(h)
                        else:
                            mm2_group(h, gi)

                if nstg:
                    pending.append((big_exp, deferred_items))
                else:
                    for fn in deferred_items:
                        fn()
            # flush any remaining deferred work (the final staged pair)
            for pb, pitems in pending:
                pb()
                for fn in pitems:
                    fn()
    nc.finalize()
    return nc


def _mask():
    j = np.arange(128)[:, None]
    i = np.arange(128)[None, :]
    return (i >= j).astype(NPBF16)                     # [j, i] allowed mask


def _mask_add():
    """Additive pre-exp mask for one 256-col strip: [tri | zeros] fp16."""
    j = np.arange(128)[:, None]
    i = np.arange(128)[None, :]
    m = np.zeros((128, 256), dtype=np.float32)
    m[:, 0:128] = np.where(i >= j, 0.0, -30000.0)
    return m.astype(NPF16)


def _rope(x):
    # x: [B', N, D] f32; global-position angles
    inv = 1.0 / THETA ** (np.arange(0, D, 2, dtype=np.float32) / D)
    ang = np.arange(N, dtype=np.float32)[:, None] * inv[None, :]   # [N, 32]
    cos = np.cos(ang)
    sin = np.sin(ang)
    lo, hi = x[..., : D // 2], x[..., D // 2:]
    out = np.empty_like(x)
    out[..., : D // 2] = lo * cos - hi * sin
    out[..., D // 2:] = hi * cos + lo * sin
    return out


def kernel(q, k, v):
    if "nc" not in _CACHE:
        _CACHE["nc"] = _build_program()
    nc = _CACHE["nc"]
    m01 = _mask()
    madd = _mask_add()

    qr = _rope(q)
    kr = _rope(k)

    in_maps = []
    for c in range(NCORES):
        sl = slice(c * BL, (c + 1) * BL)
        qc, kc, vc = qr[sl], kr[sl], v[sl]          # [16, N, 64] f32
        # d-major, batch pairs stacked on partitions: [NP, 2*64, N]
        qtc = qc.transpose(0, 2, 1).reshape(NP, 128, N)
        ktc = kc.transpose(0, 2, 1).reshape(NP, 128, N)
        # v blocked [16, 128, 32, 65] with ones column
        vbc = np.empty((BL, 128, NW, 65), dtype=NPBF16)
        vbc[..., :64] = vc.reshape(BL, NW, W, D).transpose(0, 2, 1, 3)
        vbc[..., 64] = 1.0
        in_maps.append({
            "qt": qtc.reshape(NP * 128, N).astype(NPBF16),
            "kt": ktc.reshape(NP * 128, N).astype(NPBF16),
            "vb": vbc.reshape(BL * 128, NW * 65),
            "m01": m01,
            "madd": madd,
        })

    res = run_bass_kernel_spmd(nc, in_maps, list(range(NCORES)))
    _CACHE["last_results"] = res
    out = np.empty((B, N, D), dtype=np.float32)
    for c in range(NCORES):
        ob = res.results[c]["outb"].astype(np.float32).reshape(BL, 128, NW, D)
        out[c * BL:(c + 1) * BL] = (
            ob.transpose(0, 2, 1, 3).reshape(BL, N, D)
        )
    return out


if __name__ == "__main__":
    rng = np.random.default_rng(0)
    q = rng.standard_normal((B, N, D), dtype=np.float32)
    k = rng.standard_normal((B, N, D), dtype=np.float32)
    v = rng.standard_normal((B, N, D), dtype=np.float32)
    o = kernel(q, k, v)
    print("out", o.shape, o.dtype, np.abs(o).max())
